# revision 13
# baseline (speedup 1.0000x reference)
"""Subsequence-DTW kernel for Trainium2 (8 NeuronCores, batch-parallel).

Per core (2 batches):
  C[i,j] = x2[i]+y2[j]-2 x.y produced by PE matmul (K=65 augmented contraction,
  y~ carries -y2/2), finalized C = -2*dot + x2 via tensor_scalar, DMA'd to DRAM
  in wavefront-skewed layout.
  Forward DP as block wavefront: partition p = 2*blk + batch (blk = j//64),
  wavefront t = i + blk.  One tensor_tensor_scan per wavefront does the whole
  64-wide row-block DP: state = min(u[f], state) + C[f], with the per-partition
  `initial` AP carrying D[i, blk*64-1].  The partition shift-by-2 for the carry
  runs on the PE as a shift-matrix matmul (compute engines here require
  partition-base-0 APs), which also deposits +BIG into the two batch-boundary
  partitions.  Backtrack decision planes cud/cl stored int8 in SBUF, DMA'd out
  with the final DP row; host does the O(N+M) walk + tiny gathers.
"""
import numpy as np

B, N, M, Dd = 16, 1024, 4096, 64
BLK = 64
NB = M // BLK          # 64 blocks per batch
T = N + NB - 1         # 1087 wavefronts
PITCH = 128 * BLK      # 8192 elements per wavefront slice
NCORES = 8
BPC = 2
BIG = 1.0e30

_cache = {}


def _build():
    import concourse.bacc as bacc
    import concourse.bass as bass
    import concourse.mybir as mybir
    import concourse.tile as tile

    dt = mybir.dt
    op = mybir.AluOpType
    AX = mybir.AxisListType

    nc = bacc.Bacc("TRN2", target_bir_lowering=False)

    x_in = nc.dram_tensor("x", [BPC, N, Dd], dt.float32, kind="ExternalInput")
    y_in = nc.dram_tensor("y", [BPC, M, Dd], dt.float32, kind="ExternalInput")
    ident = nc.dram_tensor("ident", [128, 128], dt.float32, kind="ExternalInput")
    shm_in = nc.dram_tensor("shiftm", [128, 128], dt.float32, kind="ExternalInput")
    b01_in = nc.dram_tensor("big01", [1, 128], dt.float32, kind="ExternalInput")
    dlast_o = nc.dram_tensor("dlast", [128, BLK], dt.float32, kind="ExternalOutput")
    cud_o = nc.dram_tensor("cud", [128, T * BLK], dt.int8, kind="ExternalOutput")
    cl_o = nc.dram_tensor("cl", [128, T * BLK], dt.int8, kind="ExternalOutput")
    cwf = nc.dram_tensor("cwf", [T * PITCH], dt.float32)

    with tile.TileContext(nc) as tc:
        with (
            tc.tile_pool(name="big", bufs=1) as bigp,
            tc.tile_pool(name="pre", bufs=1) as prep,
            tc.tile_pool(name="csb", bufs=3) as csbp,
            tc.tile_pool(name="cst", bufs=1) as cstp,
            tc.tile_pool(name="ps", bufs=1, space="PSUM") as psp,
            tc.tile_pool(name="psc", bufs=3, space="PSUM") as pscp,
            tc.tile_pool(name="pscr", bufs=2, space="PSUM") as pscrp,
            tc.tile_pool(name="dh", bufs=4) as dhp,
            tc.tile_pool(name="u", bufs=3) as up,
            tc.tile_pool(name="cin", bufs=6) as cinp,
        ):
            cud_b = bigp.tile([128, T * BLK], dt.int8, tag="cud")
            cl_b = bigp.tile([128, T * BLK], dt.int8, tag="cl")

            id_sb = cstp.tile([128, 128], dt.float32, tag="id")
            nc.sync.dma_start(id_sb[:], ident[:])
            shm_sb = cstp.tile([128, 128], dt.float32, tag="shm")
            nc.sync.dma_start(shm_sb[:], shm_in[:])
            b01_sb = cstp.tile([1, 128], dt.float32, tag="b01")
            nc.sync.dma_start(b01_sb[:], b01_in[:])
            one_sb = cstp.tile([1, 1], dt.float32, tag="one")
            nc.vector.memset(one_sb[:], 1.0)
            ones_m05 = cstp.tile([64, 1], dt.float32, tag="o5")
            nc.vector.memset(ones_m05[:], -0.5)

            # zero the leading-dead triangle of cwf (blocks not yet live)
            zrow = cstp.tile([128, BLK], dt.float32, tag="zr")
            nc.vector.memset(zrow[:], 0.0)
            for t in range(NB - 1):
                npart = 128 - 2 * t - 2
                nc.sync.dma_start(
                    bass.AP(cwf, t * PITCH + (2 * t + 2) * BLK, [[BLK, npart], [1, BLK]]),
                    zrow[0:npart, :],
                )

            # ---------------- prologue + C production (per batch) ----------
            for b in range(BPC):
                x_sb = prep.tile([128, 8 * Dd], dt.float32, tag="xsb")
                nc.sync.dma_start(
                    x_sb[:].rearrange("p (rc d) -> p rc d", d=Dd),
                    x_in[b].rearrange("(rc p) d -> p rc d", p=128),
                )
                y_sb = prep.tile([128, 32 * Dd], dt.float32, tag="ysb")
                nc.sync.dma_start(
                    y_sb[:].rearrange("p (rc d) -> p rc d", d=Dd),
                    y_in[b].rearrange("(rc p) d -> p rc d", p=128),
                )
                sqx = prep.tile([128, 8 * Dd], dt.float32, tag="sqx")
                nc.vector.tensor_tensor(sqx[:], x_sb[:], x_sb[:], op.mult)
                x2c = prep.tile([128, 8], dt.float32, tag="x2c")
                nc.vector.tensor_reduce(
                    x2c[:], sqx[:].rearrange("p (rc d) -> p rc d", d=Dd), AX.X, op.add
                )
                xT = prep.tile([65, N], dt.float32, tag="xT")
                nc.vector.memset(xT[64:65, :], 1.0)
                for rc in range(8):
                    pt = psp.tile([64, 128], dt.float32, tag="pt")
                    nc.tensor.transpose(pt[:], x_sb[:, rc * Dd:(rc + 1) * Dd], id_sb[:])
                    nc.vector.tensor_copy(xT[0:64, rc * 128:(rc + 1) * 128], pt[:])
                yT = prep.tile([65, M], dt.float32, tag="yT")
                for rc in range(32):
                    pt = psp.tile([64, 128], dt.float32, tag="pt")
                    nc.tensor.transpose(pt[:], y_sb[:, rc * Dd:(rc + 1) * Dd], id_sb[:])
                    nc.vector.tensor_copy(yT[0:64, rc * 128:(rc + 1) * 128], pt[:])
                for h in range(2):
                    sqy = prep.tile([64, M // 2], dt.float32, tag="sqy")
                    nc.vector.tensor_tensor(
                        sqy[:], yT[0:64, h * 2048:(h + 1) * 2048],
                        yT[0:64, h * 2048:(h + 1) * 2048], op.mult,
                    )
                    for c in range(4):
                        p1 = psp.tile([1, 512], dt.float32, tag="p1")
                        nc.tensor.matmul(p1[:], ones_m05[:], sqy[:, c * 512:(c + 1) * 512])
                        nc.vector.tensor_copy(
                            yT[64:65, h * 2048 + c * 512: h * 2048 + (c + 1) * 512],
                            p1[:],
                        )
                for rc in range(8):
                    for cc in range(8):
                        pc = pscp.tile([128, 512], dt.float32, tag="pc")
                        nc.tensor.matmul(
                            pc[:], xT[:, rc * 128:(rc + 1) * 128],
                            yT[:, cc * 512:(cc + 1) * 512],
                        )
                        csb = csbp.tile([128, 512], dt.float32, tag="csb")
                        nc.vector.tensor_scalar(
                            csb[:], pc[:], -2.0, x2c[:, rc:rc + 1], op.mult, op.add
                        )
                        base = (rc * 128 + cc * 8) * PITCH + cc * 8 * 128 + b * BLK
                        nc.sync.dma_start(
                            bass.AP(cwf, base, [[PITCH, 128], [PITCH + 2 * BLK, 8], [1, BLK]]),
                            csb[:].rearrange("p (a f) -> p a f", f=BLK),
                        )

            # ---------------- forward wavefront loop ------------------------
            d_init = dhp.tile([128, BLK + 1], dt.float32, tag="dh")
            nc.vector.memset(d_init[:], 0.0)
            nc.vector.memset(d_init[0:2, 0:1], BIG)
            dp = d_init

            for t in range(T):
                dcur = dhp.tile([128, BLK + 1], dt.float32, tag="dh")
                # carry: dcur[p,0] = dp[p-2, BLK], p in {0,1} get +BIG  (on PE)
                pcr = pscrp.tile([128, 1], dt.float32, tag="pcr")
                nc.tensor.matmul(pcr[:], shm_sb[:], dp[:, BLK:BLK + 1],
                                 start=True, stop=False)
                nc.tensor.matmul(pcr[:], b01_sb[:], one_sb[:],
                                 start=False, stop=True)
                nc.vector.tensor_copy(dcur[:, 0:1], pcr[:])
                # C stream in
                cin = cinp.tile([128, BLK], dt.float32, tag="cin")
                nc.sync.dma_start(
                    cin[:], bass.AP(cwf, t * PITCH, [[BLK, 128], [1, BLK]])
                )
                # u = min(Dprev, Dprev shifted one col)
                u = up.tile([128, BLK], dt.float32, tag="u")
                nc.vector.tensor_tensor(u[:], dp[:, 1:BLK + 1], dp[:, 0:BLK], op.min)
                # whole row-block DP step
                nc.vector.tensor_tensor_scan(
                    dcur[:, 1:BLK + 1], u[:], cin[:], dcur[:, 0:1], op.min, op.add
                )
                # backtrack decision planes
                nc.vector.tensor_tensor(
                    cud_b[:, t * BLK:(t + 1) * BLK],
                    dp[:, 1:BLK + 1], dp[:, 0:BLK], op.is_lt,
                )
                nc.vector.tensor_tensor(
                    cl_b[:, t * BLK:(t + 1) * BLK],
                    dcur[:, 0:BLK], u[:], op.is_lt,
                )
                # stream out the final DP row as blocks retire
                if t >= N - 1:
                    blk = t - (N - 1)
                    nc.sync.dma_start(
                        dlast_o[2 * blk:2 * blk + 2, :], dcur[2 * blk:2 * blk + 2, 1:BLK + 1]
                    )
                dp = dcur

            for q in range(4):
                s = q * (T * BLK // 4)
                e = (q + 1) * (T * BLK // 4)
                nc.sync.dma_start(cud_o[:, s:e], cud_b[:, s:e])
                nc.sync.dma_start(cl_o[:, s:e], cl_b[:, s:e])

    nc.finalize()
    return nc


def _get_nc():
    if "nc" not in _cache:
        _cache["nc"] = _build()
    return _cache["nc"]


def _walk(cud, cl, dlast, yl):
    """Host pointer walk over the decision planes (partition p = 2*blk + b)."""
    for b in range(BPC):
        dfin = dlast[b::2].reshape(-1)   # [64 blk, 64 f] -> j
        j = int(np.argmin(dfin))
        i = N - 1
        yl[b, i] = j
        cudb = cud[b::2]
        clb = cl[b::2]
        while i > 0:
            blk, f = j >> 6, j & 63
            col = (i + blk) * BLK + f
            if clb[blk, col]:
                j -= 1
            elif cudb[blk, col]:
                i -= 1
            else:
                i -= 1
                j -= 1
            if yl[b, i] < j:
                yl[b, i] = j


def kernel(x, y, x_t, y_t):
    from concourse.bass_utils import run_bass_kernel_spmd

    nc = _get_nc()
    ident = np.eye(128, dtype=np.float32)
    shiftm = np.eye(128, k=2, dtype=np.float32)   # lhsT: out[p] = in[p-2]
    big01 = np.zeros((1, 128), np.float32)
    big01[0, 0:2] = BIG
    in_maps = [
        {
            "x": np.ascontiguousarray(x[c * BPC:(c + 1) * BPC], dtype=np.float32),
            "y": np.ascontiguousarray(y[c * BPC:(c + 1) * BPC], dtype=np.float32),
            "ident": ident,
            "shiftm": shiftm,
            "big01": big01,
        }
        for c in range(NCORES)
    ]
    res = run_bass_kernel_spmd(nc, in_maps, core_ids=list(range(NCORES)))

    dtw_cost = np.zeros([B], np.float32)
    y_last = np.zeros([B, N], np.int64)
    for c in range(NCORES):
        r = res.results[c]
        dlast, cud, cl = r["dlast"], r["cud"], r["cl"]
        yl = np.zeros([BPC, N], np.int64)
        _walk(cud, cl, dlast, yl)
        for b in range(BPC):
            dtw_cost[c * BPC + b] = dlast[b::2].reshape(-1).min()
        y_last[c * BPC:(c + 1) * BPC] = yl
    w_ts = np.asarray(x_t, np.float32)
    w_vs = np.take_along_axis(np.asarray(y_t, np.float32), y_last, axis=1)
    return dtw_cost, w_ts, w_vs


# revision 34
# speedup vs baseline: 1.1194x; 1.1194x over previous
"""Subsequence-DTW kernel for Trainium2 (8 NeuronCores, batch-parallel).

Per core (2 batches):
  C[i,j] = x2[i]+y2[j]-2 x.y produced by PE matmul (K=65 augmented contraction,
  y~ carries -y2/2), finalized C = -2*dot + x2 via tensor_scalar, DMA'd to DRAM
  in wavefront-skewed layout.
  Forward DP as block wavefront: partition p = 2*blk + batch (blk = j//64),
  wavefront t = i + blk.  One tensor_tensor_scan per wavefront does the whole
  64-wide row-block DP: state = min(u[f], state) + C[f], with the per-partition
  `initial` AP carrying D[i, blk*64-1].  The partition shift-by-2 for the carry
  runs on the PE as a shift-matrix matmul (compute engines here require
  partition-base-0 APs), which also deposits +BIG into the two batch-boundary
  partitions.  Backtrack decision planes cud/cl stored int8 in SBUF, DMA'd out
  with the final DP row; host does the O(N+M) walk + tiny gathers.
"""
import numpy as np

B, N, M, Dd = 16, 1024, 4096, 64
BLK = 64
NB = M // BLK          # 64 blocks per batch
T = N + NB - 1         # 1087 wavefronts
PITCH = 128 * BLK      # 8192 elements per wavefront slice
NCORES = 8
BPC = 2
BIG = 1.0e30
CHUNK = 16             # wavefronts of C per input DMA

_cache = {}


def _build(skip_carry=False, skip_codes=False, skip_scan=False, skip_prod=False):
    import concourse.bacc as bacc
    import concourse.bass as bass
    import concourse.mybir as mybir
    import concourse.tile as tile

    dt = mybir.dt
    op = mybir.AluOpType
    AX = mybir.AxisListType

    nc = bacc.Bacc("TRN2", target_bir_lowering=False)

    x_in = nc.dram_tensor("x", [BPC, N, Dd], dt.float32, kind="ExternalInput")
    y_in = nc.dram_tensor("y", [BPC, M, Dd], dt.float32, kind="ExternalInput")
    ident = nc.dram_tensor("ident", [128, 128], dt.float32, kind="ExternalInput")
    shm_in = nc.dram_tensor("shiftm", [128, 128], dt.float32, kind="ExternalInput")
    b01_in = nc.dram_tensor("big01", [1, 128], dt.float32, kind="ExternalInput")
    dlast_o = nc.dram_tensor("dlast", [128, BLK], dt.float32, kind="ExternalOutput")
    cud_o = nc.dram_tensor("cud", [128, T * BLK], dt.int8, kind="ExternalOutput")
    cl_o = nc.dram_tensor("cl", [128, T * BLK], dt.int8, kind="ExternalOutput")
    cwf = nc.dram_tensor("cwf", [T * PITCH], dt.float32)

    with tile.TileContext(nc) as tc:
        with (
            tc.tile_pool(name="big", bufs=1) as bigp,
            tc.tile_pool(name="pre", bufs=1) as prep,
            tc.tile_pool(name="csb", bufs=3) as csbp,
            tc.tile_pool(name="cst", bufs=1) as cstp,
            tc.tile_pool(name="ps", bufs=1, space="PSUM") as psp,
            tc.tile_pool(name="psc", bufs=3, space="PSUM") as pscp,
            tc.tile_pool(name="pscr", bufs=2, space="PSUM") as pscrp,
            tc.tile_pool(name="dh", bufs=4) as dhp,
            tc.tile_pool(name="u", bufs=3) as up,
            tc.tile_pool(name="cin", bufs=2) as cinp,
        ):
            cud_b = bigp.tile([128, T * BLK], dt.int8, tag="cud")
            cl_b = bigp.tile([128, T * BLK], dt.int8, tag="cl")

            id_sb = cstp.tile([128, 128], dt.float32, tag="id")
            nc.sync.dma_start(id_sb[:], ident[:])
            shm_sb = cstp.tile([128, 128], dt.float32, tag="shm")
            nc.sync.dma_start(shm_sb[:], shm_in[:])
            b01_sb = cstp.tile([1, 128], dt.float32, tag="b01")
            nc.sync.dma_start(b01_sb[:], b01_in[:])
            one_sb = cstp.tile([1, 1], dt.float32, tag="one")
            nc.vector.memset(one_sb[:], 1.0)
            ones_m05 = cstp.tile([64, 1], dt.float32, tag="o5")
            nc.vector.memset(ones_m05[:], -0.5)

            # zero the leading-dead triangle of cwf (blocks not yet live)
            zrow = cstp.tile([128, BLK], dt.float32, tag="zr")
            nc.vector.memset(zrow[:], 0.0)
            for t in range(NB - 1):
                npart = 128 - 2 * t - 2
                nc.sync.dma_start(
                    bass.AP(cwf, t * PITCH + (2 * t + 2) * BLK, [[BLK, npart], [1, BLK]]),
                    zrow[0:npart, :],
                )

            # ---------------- prologue + C production (per batch) ----------
            def emit_prologue(b):
                x_sb = prep.tile([128, 8 * Dd], dt.float32, tag="xsb")
                nc.sync.dma_start(
                    x_sb[:].rearrange("p (rc d) -> p rc d", d=Dd),
                    x_in[b].rearrange("(rc p) d -> p rc d", p=128),
                )

                sqx = prep.tile([128, 8 * Dd], dt.float32, tag="sqx")
                nc.vector.tensor_tensor(sqx[:], x_sb[:], x_sb[:], op.mult)
                x2c = prep.tile([128, 8], dt.float32, tag=f"x2c{b}")
                nc.vector.tensor_reduce(
                    x2c[:], sqx[:].rearrange("p (rc d) -> p rc d", d=Dd), AX.X, op.add
                )
                xT = prep.tile([65, N], dt.float32r, tag=f"xT{b}")
                # memset can't target f32r: write ones via (src*0)+1
                nc.vector.tensor_scalar(
                    xT[64:65, :], cud_b[0:1, 0:N], 0.0, 1.0, op.mult, op.add
                )
                for rc in range(8):
                    pt = psp.tile([64, 128], dt.float32, tag="pt")
                    nc.tensor.transpose(pt[:], x_sb[:, rc * Dd:(rc + 1) * Dd], id_sb[:])
                    nc.vector.tensor_copy(xT[0:64, rc * 128:(rc + 1) * 128], pt[:])
                yT = prep.tile([65, M], dt.float32r, tag=f"yT{b}")
                for half in range(2):
                    y_sb = prep.tile([128, 16 * Dd], dt.float32, tag="ysb")
                    nc.sync.dma_start(
                        y_sb[:].rearrange("p (rc d) -> p rc d", d=Dd),
                        y_in[b, half * 2048:(half + 1) * 2048].rearrange(
                            "(rc p) d -> p rc d", p=128),
                    )
                    for rc in range(16):
                        g = half * 16 + rc
                        pt = psp.tile([64, 128], dt.float32, tag="pt")
                        nc.tensor.transpose(pt[:], y_sb[:, rc * Dd:(rc + 1) * Dd], id_sb[:])
                        nc.vector.tensor_copy(yT[0:64, g * 128:(g + 1) * 128], pt[:])
                for h in range(4):
                    sqy = prep.tile([64, M // 4], dt.float32, tag="sqy")
                    nc.vector.tensor_tensor(
                        sqy[:], yT[0:64, h * 1024:(h + 1) * 1024],
                        yT[0:64, h * 1024:(h + 1) * 1024], op.mult,
                    )
                    for c in range(2):
                        p1 = psp.tile([1, 512], dt.float32, tag="p1")
                        nc.tensor.matmul(p1[:], ones_m05[:], sqy[:, c * 512:(c + 1) * 512])
                        nc.vector.tensor_copy(
                            yT[64:65, h * 1024 + c * 512: h * 1024 + (c + 1) * 512],
                            p1[:],
                        )
                return xT, yT, x2c

            def emit_band(b, xT, yT, x2c, rc):
                for cc in range(8):
                    pc = pscp.tile([128, 512], dt.float32, tag="pc")
                    nc.tensor.matmul(
                        pc[:], xT[:, rc * 128:(rc + 1) * 128],
                        yT[:, cc * 512:(cc + 1) * 512],
                    )
                    csb = csbp.tile([128, 512], dt.float32, tag="csb")
                    nc.vector.tensor_scalar(
                        csb[:], pc[:], -2.0, x2c[:, rc:rc + 1], op.mult, op.add
                    )
                    base = (rc * 128 + cc * 8) * PITCH + cc * 8 * 128 + b * BLK
                    nc.sync.dma_start(
                        bass.AP(cwf, base, [[PITCH, 128], [PITCH + 2 * BLK, 8], [1, BLK]]),
                        csb[:].rearrange("p (a f) -> p a f", f=BLK),
                    )

            # ---------------- forward wavefront loop ------------------------
            d_init = dhp.tile([128, BLK + 1], dt.float32, tag="dh")
            nc.vector.memset(d_init[:], 0.0)
            nc.vector.memset(d_init[0:2, 0:1], BIG)
            st = {"dp": d_init, "cin_c": None}

            def emit_wavefront(t):
                dp = st["dp"]
                cin_c = st["cin_c"]
                dcur = dhp.tile([128, BLK + 1], dt.float32, tag="dh")
                # carry: dcur[p,0] = dp[p-2, BLK], p in {0,1} get +BIG  (on PE)
                pcr = pscrp.tile([128, 1], dt.float32, tag="pcr")
                if not skip_carry:
                    # constant BIG-injector first: it has no data deps, so PE
                    # runs it ahead; only the shift matmul sits on the chain
                    nc.tensor.matmul(pcr[:], b01_sb[:], one_sb[:],
                                     start=True, stop=False)
                    nc.tensor.matmul(pcr[:], shm_sb[:], dp[:, BLK:BLK + 1],
                                     start=False, stop=True)
                    # off-chain: materialize carry column for next-wf u / cl
                    nc.vector.tensor_copy(dcur[:, 0:1], pcr[:])
                # C stream in: one strided DMA covers CHUNK wavefronts
                if t % CHUNK == 0:
                    nw = min(CHUNK, T - t)
                    cin_c = cinp.tile([128, CHUNK * BLK], dt.float32, tag="cin")
                    nc.sync.dma_start(
                        cin_c[:, 0:nw * BLK].rearrange("p (k f) -> p k f", f=BLK),
                        bass.AP(cwf, t * PITCH, [[BLK, 128], [PITCH, nw], [1, BLK]]),
                    )
                st["cin_c"] = cin_c
                cin = cin_c[:, (t % CHUNK) * BLK:(t % CHUNK + 1) * BLK]
                # u = min(Dprev, Dprev shifted one col)
                u = up.tile([128, BLK], dt.float32, tag="u")
                nc.vector.tensor_tensor(u[:], dp[:, 1:BLK + 1], dp[:, 0:BLK], op.min)
                # whole row-block DP step; initial read straight from PSUM so
                # the chain is scan -> PE matmul -> scan
                if not skip_scan:
                    nc.vector.tensor_tensor_scan(
                        dcur[:, 1:BLK + 1], u[:], cin,
                        pcr[:] if not skip_carry else BIG, op.min, op.add
                    )
                # backtrack decision plane: code = 2*(dl<min(dd,du)) + (du<dd)
                if not skip_codes:
                    nc.vector.tensor_tensor(
                        cud_b[:, t * BLK:(t + 1) * BLK],
                        dp[:, 1:BLK + 1], dp[:, 0:BLK], op.is_lt,
                    )
                    nc.vector.tensor_tensor(
                        cl_b[:, t * BLK:(t + 1) * BLK],
                        dcur[:, 0:BLK], u[:], op.is_lt,
                    )
                # stream out the final DP row as blocks retire
                if t >= N - 1:
                    blk = t - (N - 1)
                    nc.sync.dma_start(
                        dlast_o[2 * blk:2 * blk + 2, :], dcur[2 * blk:2 * blk + 2, 1:BLK + 1]
                    )
                st["dp"] = dcur

            # batch 0 C fully produced up front; batch 1 bands interleave with
            # the wavefronts that consume them (wavefront t needs rows <= t,
            # i.e. bands <= t//128 of both batches)
            pro = [emit_prologue(0), emit_prologue(1)]
            for rc in range(8):
                for b in range(BPC):
                    emit_band(b, *pro[b], rc)
                for t in range(rc * 128, (rc + 1) * 128):
                    emit_wavefront(t)
            for t in range(N, T):
                emit_wavefront(t)

            for q in range(4):
                s = q * (T * BLK // 4)
                e = (q + 1) * (T * BLK // 4)
                nc.sync.dma_start(cud_o[:, s:e], cud_b[:, s:e])
                nc.sync.dma_start(cl_o[:, s:e], cl_b[:, s:e])

    nc.finalize()
    return nc


def _get_nc():
    if "nc" not in _cache:
        _cache["nc"] = _build()
    return _cache["nc"]


def _walk(cud, cl, dlast, yl):
    """Host pointer walk over the decision planes (partition p = 2*blk + b)."""
    for b in range(BPC):
        dfin = dlast[b::2].reshape(-1)   # [64 blk, 64 f] -> j
        j = int(np.argmin(dfin))
        i = N - 1
        yl[b, i] = j
        cudb = cud[b::2]
        clb = cl[b::2]
        while i > 0:
            blk, f = j >> 6, j & 63
            col = (i + blk) * BLK + f
            if clb[blk, col]:
                j -= 1
            elif cudb[blk, col]:
                i -= 1
            else:
                i -= 1
                j -= 1
            if yl[b, i] < j:
                yl[b, i] = j


def kernel(x, y, x_t, y_t):
    from concourse.bass_utils import run_bass_kernel_spmd

    nc = _get_nc()
    ident = np.eye(128, dtype=np.float32)
    shiftm = np.eye(128, k=2, dtype=np.float32)   # lhsT: out[p] = in[p-2]
    big01 = np.zeros((1, 128), np.float32)
    big01[0, 0:2] = BIG
    in_maps = [
        {
            "x": np.ascontiguousarray(x[c * BPC:(c + 1) * BPC], dtype=np.float32),
            "y": np.ascontiguousarray(y[c * BPC:(c + 1) * BPC], dtype=np.float32),
            "ident": ident,
            "shiftm": shiftm,
            "big01": big01,
        }
        for c in range(NCORES)
    ]
    res = run_bass_kernel_spmd(nc, in_maps, core_ids=list(range(NCORES)))

    dtw_cost = np.zeros([B], np.float32)
    y_last = np.zeros([B, N], np.int64)
    for c in range(NCORES):
        r = res.results[c]
        dlast, cud, cl = r["dlast"], r["cud"], r["cl"]
        yl = np.zeros([BPC, N], np.int64)
        _walk(cud, cl, dlast, yl)
        for b in range(BPC):
            dtw_cost[c * BPC + b] = dlast[b::2].reshape(-1).min()
        y_last[c * BPC:(c + 1) * BPC] = yl
    w_ts = np.asarray(x_t, np.float32)
    w_vs = np.take_along_axis(np.asarray(y_t, np.float32), y_last, axis=1)
    return dtw_cost, w_ts, w_vs
